# revision 17
# baseline (speedup 1.0000x reference)
"""nn_GateModLinear on 8 trn2 cores, data-parallel over batch.

  h[b,m,i] = sum_j Ws[m,i,j] x[b,j]
  z = gW * sum_m pW[b,m] h[b,m,:] + gb * (pb @ bs)
  out = ELU(LayerNorm(z))

Sharding: batch 4096 -> 8 cores x 512 rows. Ws/bs replicated.
Device kernel (per core): bf16 matmuls with x^T stationary and W^T
streaming from HBM once; expert mix fused into the PSUM eviction
(per-partition scalar multiply-accumulate); LN via bn_stats; ELU
composed as relu(y) + min(exp(y),1) - 1.
"""

import numpy as np
import ml_dtypes

B, M, DI, DO = 4096, 8, 2048, 2048
NCORES = 8
BLOC = B // NCORES          # 512 batch rows per core
LN_EPS = 1e-5

P = 128                     # partitions
NBT = BLOC // P             # 4 batch tiles per core
NIC = DO // 512             # 4 output chunks of 512
NJT = DI // P               # 16 contraction tiles

_CACHE = {}


def _to_bf16(a):
    """fp32 (contiguous) -> bf16 with round-to-nearest-even, vectorized."""
    a = np.ascontiguousarray(a, np.float32)
    v = a.view(np.uint32)
    out = ((v + 0x7FFF + ((v >> 16) & 1)) >> 16).astype(np.uint16)
    return out.view(ml_dtypes.bfloat16)


def _build():
    import concourse.bass as bass
    import concourse.mybir as mybir
    import concourse.tile as tile
    from concourse import bacc

    f32 = mybir.dt.float32
    bf16 = mybir.dt.bfloat16
    Alu = mybir.AluOpType
    Act = mybir.ActivationFunctionType

    nc = bacc.Bacc("TRN2")

    # W2p[(m*NIC+ic)*NWC+c, p, jp*512+i] = Ws[m, ic*512+i, (c*JPC+jp)*128+p]
    W2 = nc.dram_tensor(
        "W2", [M * (DO // 512) * 4, P, (DI // P // 4) * 512], bf16,
        kind="ExternalInput",
    )
    # xTp[bt, p, jt*128+b] = x[bt*128+b, jt*128+p]  (per-core rows)
    xT = nc.dram_tensor("xT", [BLOC // P, P, DI], bf16, kind="ExternalInput")
    pW = nc.dram_tensor("pW", [BLOC, M], f32, kind="ExternalInput")
    pbT = nc.dram_tensor("pbT", [M, BLOC], bf16, kind="ExternalInput")
    bs = nc.dram_tensor("bs", [M, DO], bf16, kind="ExternalInput")
    gW = nc.dram_tensor("gW", [BLOC, DO], f32, kind="ExternalInput")
    gb = nc.dram_tensor("gb", [BLOC, DO], f32, kind="ExternalInput")
    out = nc.dram_tensor("out", [BLOC, DO], f32, kind="ExternalOutput")

    NWC = 4                     # j-tile chunks per weight slab
    JPC = NJT // NWC            # j tiles per chunk (4)

    with tile.TileContext(nc) as tc:
        with (
            tc.tile_pool(name="singles", bufs=1) as singles,
            tc.tile_pool(name="wpool", bufs=2 * NWC) as wpool,
            tc.tile_pool(name="gpool", bufs=3) as gpool,
            tc.tile_pool(name="spool", bufs=4) as spool,
            tc.tile_pool(name="epool", bufs=2) as epool,
            tc.tile_pool(name="psum_h", bufs=6, space="PSUM") as psum_h,
            tc.tile_pool(name="psum_b", bufs=2, space="PSUM") as psum_b,
        ):
            # ---- input loads; xT block 0 first so the first main matmul
            # group can start as early as possible ----
            xT_sb = []
            for bt in range(NBT):
                t = singles.tile([P, NJT, P], bf16, name=f"xT{bt}", tag=f"xT{bt}")
                nc.scalar.dma_start(out=t[:], in_=xT[bt])
                xT_sb.append(t)
            pbT_sb = singles.tile([M, BLOC], bf16)
            nc.scalar.dma_start(out=pbT_sb[:], in_=pbT[:])
            bs_sb = singles.tile([M, DO], bf16)
            nc.scalar.dma_start(out=bs_sb[:], in_=bs[:])
            pw_sb = singles.tile([P, NBT, M], f32)
            nc.scalar.dma_start(
                out=pw_sb[:], in_=pW[:].rearrange("(bt p) m -> p bt m", p=P)
            )
            eps_sb = singles.tile([P, 1], f32)
            nc.vector.memset(eps_sb[:], LN_EPS)

            # z accumulator, bias product, LN stats per batch row-block
            z_sb = [
                singles.tile([P, DO], f32, name=f"z{bt}", tag=f"z{bt}")
                for bt in range(NBT)
            ]
            pbs_sb = [
                singles.tile([P, DO], f32, name=f"pbs{bt}", tag=f"pbs{bt}")
                for bt in range(NBT)
            ]
            st_sb = [
                singles.tile([P, NIC, 6], f32, name=f"st{bt}", tag=f"st{bt}")
                for bt in range(NBT)
            ]

            def emit_bias_matmuls():
                # pbs = pb @ bs (bf16, K=8); only needed by the gating at the
                # end of the first output chunk, so these slot in after the
                # first main matmul group
                for bt in range(NBT):
                    for ic in range(NIC):
                        pb_ps = psum_b.tile([P, 512], f32, name="pb_ps")
                        nc.tensor.matmul(
                            pb_ps[:],
                            pbT_sb[:, bt * P : (bt + 1) * P],
                            bs_sb[:, ic * 512 : (ic + 1) * 512],
                        )
                        nc.scalar.copy(
                            pbs_sb[bt][:, ic * 512 : (ic + 1) * 512], pb_ps[:]
                        )

            # ---- main: h matmuls + fused expert mix ----
            for ic in range(NIC):
                for m in range(M):
                    # weight slab for (m, ic), split into NWC contiguous
                    # chunk tiles (host packed to SBUF layout)
                    w_ch = []
                    for c in range(NWC):
                        w = wpool.tile([P, JPC, 512], bf16, name=f"w{c}", tag="w")
                        nc.sync.dma_start(
                            out=w[:], in_=W2[(m * NIC + ic) * NWC + c]
                        )
                        w_ch.append(w)
                    for bt in range(NBT):
                        ph = psum_h.tile([P, 512], f32)
                        for jt in range(NJT):
                            nc.tensor.matmul(
                                ph[:],
                                xT_sb[bt][:, jt, :],
                                w_ch[jt // JPC][:, jt % JPC, :],
                                start=(jt == 0),
                                stop=(jt == NJT - 1),
                            )
                        zslab = z_sb[bt][:, ic * 512 : (ic + 1) * 512]
                        if m == 0:
                            # z = pW[:,0] * h0   (ACT, per-partition scale)
                            nc.scalar.mul(zslab, ph[:], pw_sb[:, bt, 0:1])
                        else:
                            # z = pW[:,m] * h_m + z   (DVE, fused)
                            nc.vector.scalar_tensor_tensor(
                                out=zslab,
                                in0=ph[:],
                                scalar=pw_sb[:, bt, m : m + 1],
                                in1=zslab,
                                op0=Alu.mult,
                                op1=Alu.add,
                            )
                    if ic == 0 and m == 0:
                        emit_bias_matmuls()
                # gating for this output chunk: z = gW*z + gb*pbs, then the
                # slab's LN partial stats
                for bt in range(NBT):
                    cs = slice(ic * 512, (ic + 1) * 512)
                    gw_t = gpool.tile([P, 512], f32)
                    nc.scalar.dma_start(out=gw_t[:], in_=gW[bt * P : (bt + 1) * P, cs])
                    gb_t = gpool.tile([P, 512], f32)
                    nc.scalar.dma_start(out=gb_t[:], in_=gb[bt * P : (bt + 1) * P, cs])
                    u_t = gpool.tile([P, 512], f32)
                    nc.vector.tensor_mul(u_t[:], gb_t[:], pbs_sb[bt][:, cs])
                    nc.vector.tensor_mul(z_sb[bt][:, cs], z_sb[bt][:, cs], gw_t[:])
                    nc.vector.tensor_add(z_sb[bt][:, cs], z_sb[bt][:, cs], u_t[:])
                    nc.vector.bn_stats(out=st_sb[bt][:, ic, :], in_=z_sb[bt][:, cs])

            # ---- epilogue: LayerNorm + ELU + store ----
            # one Sqrt over all four row-blocks to avoid ACT table thrash
            mv = spool.tile([P, NBT, 2], f32, name="mv")
            rstd = spool.tile([P, NBT], f32, name="rstd")
            bln = spool.tile([P, NBT], f32, name="bln")
            for bt in range(NBT):
                nc.vector.bn_aggr(out=mv[:, bt, :], in_=st_sb[bt][:])
            nc.scalar.activation(
                out=rstd[:], in_=mv[:, :, 1], func=Act.Sqrt, bias=eps_sb[:]
            )
            nc.vector.reciprocal(out=rstd[:], in_=rstd[:])
            # bln = -mean * rstd
            nc.vector.tensor_mul(bln[:], mv[:, :, 0], rstd[:])
            nc.vector.tensor_scalar_mul(bln[:], bln[:], -1.0)
            for bt in range(NBT):
                zrow = z_sb[bt][:]                      # [128, 2048]
                # y = z * rstd + bln  (LayerNorm, on ACT)
                nc.scalar.activation(
                    out=zrow,
                    in_=zrow,
                    func=Act.Identity,
                    scale=rstd[:, bt : bt + 1],
                    bias=bln[:, bt : bt + 1],
                )
                # ELU(y) = exp(min(y,0)) - 1 + max(y,0)
                t_t = epool.tile([P, DO], f32)
                nc.vector.tensor_scalar_min(t_t[:], zrow, 0.0)
                e_t = epool.tile([P, DO], f32)
                nc.scalar.activation(out=e_t[:], in_=t_t[:], func=Act.Exp)
                r_t = epool.tile([P, DO], f32)
                nc.vector.tensor_scalar_max(r_t[:], zrow, 0.0)
                nc.vector.scalar_tensor_tensor(
                    out=zrow,
                    in0=e_t[:],
                    scalar=1.0,
                    in1=r_t[:],
                    op0=Alu.subtract,
                    op1=Alu.add,
                )
                nc.scalar.dma_start(out=out[bt * P : (bt + 1) * P, :], in_=zrow)

    nc.compile()
    return nc


def _get_nc():
    if "nc" not in _CACHE:
        _CACHE["nc"] = _build()
    return _CACHE["nc"]


def _make_in_maps(x, Ws, bs, pW, pb, gW, gb):
    x = np.ascontiguousarray(x, np.float32)
    Ws = np.asarray(Ws, np.float32)
    bs = np.ascontiguousarray(bs, np.float32)
    pW = np.ascontiguousarray(pW, np.float32)
    pb = np.ascontiguousarray(pb, np.float32)
    gW = np.ascontiguousarray(gW, np.float32)
    gb = np.ascontiguousarray(gb, np.float32)

    NWC, JPC, NIC_, NJT_ = 4, DI // P // 4, DO // 512, DI // P

    # pack W to the device SBUF layout so every weight-chunk DMA is fully
    # contiguous:  W2p[(m*NIC+ic)*NWC+c, p, jp*512+i] = Ws[m, ic*512+i,
    # (c*JPC+jp)*128+p]
    Wb = _to_bf16(Ws)                                   # [M, DO, DI] bf16
    W2p = np.ascontiguousarray(
        Wb.reshape(M, NIC_, 512, NWC, JPC, P).transpose(0, 1, 3, 5, 4, 2)
    ).reshape(M * NIC_ * NWC, P, JPC * 512)

    xb = _to_bf16(x)                                    # [B, DI] bf16
    pbT = np.ascontiguousarray(_to_bf16(pb).T)          # [M, B] bf16
    bs16 = _to_bf16(bs)                                 # [M, DO] bf16

    in_maps = []
    for c in range(NCORES):
        sl = slice(c * BLOC, (c + 1) * BLOC)
        # xTp[bt, p, jt*128+b] = x[c*512 + bt*128+b, jt*128+p]
        xc = xb[sl]                                     # [512, DI]
        xTp = np.ascontiguousarray(
            xc.reshape(NBT, P, NJT_, P).transpose(0, 3, 2, 1)
        ).reshape(NBT, P, DI)
        in_maps.append(
            {
                "W2": W2p,
                "xT": xTp,
                "pW": pW[sl],
                "pbT": np.ascontiguousarray(pbT[:, sl]),
                "bs": bs16,
                "gW": gW[sl],
                "gb": gb[sl],
            }
        )
    return in_maps


def _run(in_maps, trace=False, tmpdir=None):
    from concourse.bass_utils import run_bass_kernel_spmd

    nc = _get_nc()
    res = run_bass_kernel_spmd(
        nc, in_maps, list(range(NCORES)), trace=trace, tmpdir=tmpdir
    )
    out = np.concatenate([r["out"] for r in res.results], axis=0)
    return out, res


def kernel(x, Ws, bs, pW, pb, gW, gb):
    in_maps = _make_in_maps(x, Ws, bs, pW, pb, gW, gb)
    out, _ = _run(in_maps)
    return out


def _install_ntff_hook():
    """Provide antenv.axon_hooks (absent in this image) so that
    run_bass_kernel_spmd(trace=True) can NTFF-profile via the axon .so."""
    try:
        from antenv.axon_hooks import get_axon_ntff_profile_hook  # noqa: F401

        return
    except ImportError:
        pass

    import contextlib
    import ctypes
    import sys
    import types

    so_path = "/opt/axon/libaxon_pjrt.so"
    lib = ctypes.CDLL(so_path)
    if not hasattr(lib, "axon_start_nrt_profile"):
        return
    lib.axon_start_nrt_profile.argtypes = [
        ctypes.POINTER(ctypes.c_int64),
        ctypes.c_size_t,
    ]
    lib.axon_start_nrt_profile.restype = ctypes.c_int64
    lib.axon_stop_nrt_profile.argtypes = [ctypes.c_char_p]
    lib.axon_stop_nrt_profile.restype = ctypes.c_int64

    @contextlib.contextmanager
    def _hook(output_dir, device_ids):
        import jax

        jax.devices()
        if device_ids:
            ids = (ctypes.c_int64 * len(device_ids))(*device_ids)
            rc = lib.axon_start_nrt_profile(ids, len(device_ids))
        else:
            rc = lib.axon_start_nrt_profile(None, 0)
        if rc != 0:
            raise RuntimeError(f"axon_start_nrt_profile rc={rc}")
        try:
            yield
        finally:
            n = lib.axon_stop_nrt_profile(str(output_dir).encode())
            print(f"[ntff] {n} file(s) written to {output_dir}")

    mod = types.ModuleType("antenv.axon_hooks")
    mod._hook = _hook
    mod.get_axon_ntff_profile_hook = lambda: mod._hook
    mod.set_axon_ntff_profile_hook = lambda h: setattr(mod, "_hook", h)
    sys.modules["antenv.axon_hooks"] = mod


def kernel_traced(x, Ws, bs, pW, pb, gW, gb, tmpdir=None):
    """Like kernel(), also returns the NTFF-profiled HW exec time (ns)."""
    _install_ntff_hook()
    in_maps = _make_in_maps(x, Ws, bs, pW, pb, gW, gb)
    out, res = _run(in_maps, trace=True, tmpdir=tmpdir)
    return out, res.exec_time_ns


# revision 18
# speedup vs baseline: 1.0179x; 1.0179x over previous
"""nn_GateModLinear on 8 trn2 cores, data-parallel over batch.

  h[b,m,i] = sum_j Ws[m,i,j] x[b,j]
  z = gW * sum_m pW[b,m] h[b,m,:] + gb * (pb @ bs)
  out = ELU(LayerNorm(z))

Sharding: batch 4096 -> 8 cores x 512 rows. Ws/bs replicated.
Device kernel (per core): bf16 matmuls with x^T stationary and W^T
streaming from HBM once; expert mix fused into the PSUM eviction
(per-partition scalar multiply-accumulate); LN via bn_stats; ELU
composed as relu(y) + min(exp(y),1) - 1.
"""

import numpy as np
import ml_dtypes

B, M, DI, DO = 4096, 8, 2048, 2048
NCORES = 8
BLOC = B // NCORES          # 512 batch rows per core
LN_EPS = 1e-5

P = 128                     # partitions
NBT = BLOC // P             # 4 batch tiles per core
NIC = DO // 512             # 4 output chunks of 512
NJT = DI // P               # 16 contraction tiles

_CACHE = {}


def _to_bf16(a):
    """fp32 (contiguous) -> bf16 with round-to-nearest-even, vectorized."""
    a = np.ascontiguousarray(a, np.float32)
    v = a.view(np.uint32)
    out = ((v + 0x7FFF + ((v >> 16) & 1)) >> 16).astype(np.uint16)
    return out.view(ml_dtypes.bfloat16)


def _build():
    import concourse.bass as bass
    import concourse.mybir as mybir
    import concourse.tile as tile
    from concourse import bacc

    f32 = mybir.dt.float32
    bf16 = mybir.dt.bfloat16
    Alu = mybir.AluOpType
    Act = mybir.ActivationFunctionType

    nc = bacc.Bacc("TRN2")

    # W2p[(m*NIC+ic)*NWC+c, p, jp*512+i] = Ws[m, ic*512+i, (c*JPC+jp)*128+p]
    W2 = nc.dram_tensor(
        "W2", [M * (DO // 512) * 4, P, (DI // P // 4) * 512], bf16,
        kind="ExternalInput",
    )
    # xTp[bt, p, jt*128+b] = x[bt*128+b, jt*128+p]  (per-core rows)
    xT = nc.dram_tensor("xT", [BLOC // P, P, DI], bf16, kind="ExternalInput")
    pW = nc.dram_tensor("pW", [BLOC, M], f32, kind="ExternalInput")
    pbT = nc.dram_tensor("pbT", [M, BLOC], bf16, kind="ExternalInput")
    bs = nc.dram_tensor("bs", [M, DO], bf16, kind="ExternalInput")
    gW = nc.dram_tensor("gW", [BLOC, DO], f32, kind="ExternalInput")
    gb = nc.dram_tensor("gb", [BLOC, DO], f32, kind="ExternalInput")
    out = nc.dram_tensor("out", [BLOC, DO], f32, kind="ExternalOutput")

    NWC = 4                     # j-tile chunks per weight slab
    JPC = NJT // NWC            # j tiles per chunk (4)

    with tile.TileContext(nc) as tc:
        with (
            tc.tile_pool(name="singles", bufs=1) as singles,
            tc.tile_pool(name="wpool", bufs=2 * NWC) as wpool,
            tc.tile_pool(name="gpool", bufs=3) as gpool,
            tc.tile_pool(name="spool", bufs=4) as spool,
            tc.tile_pool(name="epool", bufs=2) as epool,
            tc.tile_pool(name="psum_h", bufs=6, space="PSUM") as psum_h,
            tc.tile_pool(name="psum_b", bufs=2, space="PSUM") as psum_b,
        ):
            # ---- input loads; xT block 0 first so the first main matmul
            # group can start as early as possible ----
            xT_sb = []
            for bt in range(NBT):
                t = singles.tile([P, NJT, P], bf16, name=f"xT{bt}", tag=f"xT{bt}")
                nc.scalar.dma_start(out=t[:], in_=xT[bt])
                xT_sb.append(t)
            pbT_sb = singles.tile([M, BLOC], bf16)
            nc.scalar.dma_start(out=pbT_sb[:], in_=pbT[:])
            bs_sb = singles.tile([M, DO], bf16)
            nc.scalar.dma_start(out=bs_sb[:], in_=bs[:])
            pw_sb = singles.tile([P, NBT, M], f32)
            nc.scalar.dma_start(
                out=pw_sb[:], in_=pW[:].rearrange("(bt p) m -> p bt m", p=P)
            )
            eps_sb = singles.tile([P, 1], f32)
            nc.vector.memset(eps_sb[:], LN_EPS)

            # z accumulator, bias product, LN stats per batch row-block
            z_sb = [
                singles.tile([P, DO], f32, name=f"z{bt}", tag=f"z{bt}")
                for bt in range(NBT)
            ]
            pbs_sb = [
                singles.tile([P, DO], f32, name=f"pbs{bt}", tag=f"pbs{bt}")
                for bt in range(NBT)
            ]
            st_sb = [
                singles.tile([P, NIC, 6], f32, name=f"st{bt}", tag=f"st{bt}")
                for bt in range(NBT)
            ]

            def emit_bias_matmuls():
                # pbs = pb @ bs (bf16, K=8); only needed by the gating at the
                # end of the first output chunk, so these slot in after the
                # first main matmul group
                for bt in range(NBT):
                    for ic in range(NIC):
                        pb_ps = psum_b.tile([P, 512], f32, name="pb_ps")
                        nc.tensor.matmul(
                            pb_ps[:],
                            pbT_sb[:, bt * P : (bt + 1) * P],
                            bs_sb[:, ic * 512 : (ic + 1) * 512],
                        )
                        nc.scalar.copy(
                            pbs_sb[bt][:, ic * 512 : (ic + 1) * 512], pb_ps[:]
                        )

            # ---- main: h matmuls + fused expert mix ----
            for ic in range(NIC):
                for m in range(M):
                    # weight slab for (m, ic), split into NWC contiguous
                    # chunk tiles (host packed to SBUF layout)
                    w_ch = []
                    for c in range(NWC):
                        w = wpool.tile([P, JPC, 512], bf16, name=f"w{c}", tag="w")
                        nc.sync.dma_start(
                            out=w[:], in_=W2[(m * NIC + ic) * NWC + c]
                        )
                        w_ch.append(w)
                    for bt in range(NBT):
                        ph = psum_h.tile([P, 512], f32)
                        for jt in range(NJT):
                            nc.tensor.matmul(
                                ph[:],
                                xT_sb[bt][:, jt, :],
                                w_ch[jt // JPC][:, jt % JPC, :],
                                start=(jt == 0),
                                stop=(jt == NJT - 1),
                            )
                        zslab = z_sb[bt][:, ic * 512 : (ic + 1) * 512]
                        if m == 0:
                            # z = pW[:,0] * h0   (ACT, per-partition scale)
                            nc.scalar.mul(zslab, ph[:], pw_sb[:, bt, 0:1])
                        else:
                            # z = pW[:,m] * h_m + z   (DVE, fused)
                            nc.vector.scalar_tensor_tensor(
                                out=zslab,
                                in0=ph[:],
                                scalar=pw_sb[:, bt, m : m + 1],
                                in1=zslab,
                                op0=Alu.mult,
                                op1=Alu.add,
                            )
                    if ic == 0 and m == 0:
                        emit_bias_matmuls()
                # gating for this output chunk: z = gW*z + gb*pbs, then the
                # slab's LN partial stats
                for bt in range(NBT):
                    cs = slice(ic * 512, (ic + 1) * 512)
                    gw_t = gpool.tile([P, 512], f32)
                    nc.scalar.dma_start(out=gw_t[:], in_=gW[bt * P : (bt + 1) * P, cs])
                    gb_t = gpool.tile([P, 512], f32)
                    nc.scalar.dma_start(out=gb_t[:], in_=gb[bt * P : (bt + 1) * P, cs])
                    u_t = gpool.tile([P, 512], f32)
                    nc.vector.tensor_mul(u_t[:], gb_t[:], pbs_sb[bt][:, cs])
                    nc.vector.tensor_mul(z_sb[bt][:, cs], z_sb[bt][:, cs], gw_t[:])
                    nc.vector.tensor_add(z_sb[bt][:, cs], z_sb[bt][:, cs], u_t[:])
                    nc.vector.bn_stats(out=st_sb[bt][:, ic, :], in_=z_sb[bt][:, cs])

            # ---- epilogue: LayerNorm + ELU + store ----
            # one Sqrt over all four row-blocks to avoid ACT table thrash
            mv = spool.tile([P, NBT, 2], f32, name="mv")
            rstd = spool.tile([P, NBT], f32, name="rstd")
            bln = spool.tile([P, NBT], f32, name="bln")
            for bt in range(NBT):
                nc.vector.bn_aggr(out=mv[:, bt, :], in_=st_sb[bt][:])
            nc.scalar.activation(
                out=rstd[:], in_=mv[:, :, 1], func=Act.Sqrt, bias=eps_sb[:]
            )
            nc.vector.reciprocal(out=rstd[:], in_=rstd[:])
            # bln = -mean * rstd
            nc.vector.tensor_mul(bln[:], mv[:, :, 0], rstd[:])
            nc.vector.tensor_scalar_mul(bln[:], bln[:], -1.0)
            # y = z*rstd + bln is LN output, |y| < ~sqrt(DO) so exp(y) is
            # finite in fp32.  ELU(y) = min(exp(y), 1) + (max(y, 0) - 1).
            for bt in range(NBT):
                zrow = z_sb[bt][:]                      # [128, 2048]
                # e = exp(z*rstd + bln)  -- LN fused into the ACT affine
                e_t = epool.tile([P, DO], f32)
                nc.scalar.activation(
                    out=e_t[:],
                    in_=zrow,
                    func=Act.Exp,
                    scale=rstd[:, bt : bt + 1],
                    bias=bln[:, bt : bt + 1],
                )
                # y = z*rstd + bln  (DVE, 2x mode)
                y_t = epool.tile([P, DO], f32)
                nc.vector.tensor_scalar(
                    out=y_t[:],
                    in0=zrow,
                    scalar1=rstd[:, bt : bt + 1],
                    scalar2=bln[:, bt : bt + 1],
                    op0=Alu.mult,
                    op1=Alu.add,
                )
                # r = max(y,0) - 1  (DVE, 2x mode)
                nc.vector.tensor_scalar(
                    out=y_t[:],
                    in0=y_t[:],
                    scalar1=0.0,
                    scalar2=1.0,
                    op0=Alu.max,
                    op1=Alu.subtract,
                )
                # out = min(e,1) + r
                nc.vector.scalar_tensor_tensor(
                    out=zrow,
                    in0=e_t[:],
                    scalar=1.0,
                    in1=y_t[:],
                    op0=Alu.min,
                    op1=Alu.add,
                )
                nc.scalar.dma_start(out=out[bt * P : (bt + 1) * P, :], in_=zrow)

    nc.compile()
    return nc


def _get_nc():
    if "nc" not in _CACHE:
        _CACHE["nc"] = _build()
    return _CACHE["nc"]


def _make_in_maps(x, Ws, bs, pW, pb, gW, gb):
    x = np.ascontiguousarray(x, np.float32)
    Ws = np.asarray(Ws, np.float32)
    bs = np.ascontiguousarray(bs, np.float32)
    pW = np.ascontiguousarray(pW, np.float32)
    pb = np.ascontiguousarray(pb, np.float32)
    gW = np.ascontiguousarray(gW, np.float32)
    gb = np.ascontiguousarray(gb, np.float32)

    NWC, JPC, NIC_, NJT_ = 4, DI // P // 4, DO // 512, DI // P

    # pack W to the device SBUF layout so every weight-chunk DMA is fully
    # contiguous:  W2p[(m*NIC+ic)*NWC+c, p, jp*512+i] = Ws[m, ic*512+i,
    # (c*JPC+jp)*128+p]
    Wb = _to_bf16(Ws)                                   # [M, DO, DI] bf16
    W2p = np.ascontiguousarray(
        Wb.reshape(M, NIC_, 512, NWC, JPC, P).transpose(0, 1, 3, 5, 4, 2)
    ).reshape(M * NIC_ * NWC, P, JPC * 512)

    xb = _to_bf16(x)                                    # [B, DI] bf16
    pbT = np.ascontiguousarray(_to_bf16(pb).T)          # [M, B] bf16
    bs16 = _to_bf16(bs)                                 # [M, DO] bf16

    in_maps = []
    for c in range(NCORES):
        sl = slice(c * BLOC, (c + 1) * BLOC)
        # xTp[bt, p, jt*128+b] = x[c*512 + bt*128+b, jt*128+p]
        xc = xb[sl]                                     # [512, DI]
        xTp = np.ascontiguousarray(
            xc.reshape(NBT, P, NJT_, P).transpose(0, 3, 2, 1)
        ).reshape(NBT, P, DI)
        in_maps.append(
            {
                "W2": W2p,
                "xT": xTp,
                "pW": pW[sl],
                "pbT": np.ascontiguousarray(pbT[:, sl]),
                "bs": bs16,
                "gW": gW[sl],
                "gb": gb[sl],
            }
        )
    return in_maps


def _run(in_maps, trace=False, tmpdir=None):
    from concourse.bass_utils import run_bass_kernel_spmd

    nc = _get_nc()
    res = run_bass_kernel_spmd(
        nc, in_maps, list(range(NCORES)), trace=trace, tmpdir=tmpdir
    )
    out = np.concatenate([r["out"] for r in res.results], axis=0)
    return out, res


def kernel(x, Ws, bs, pW, pb, gW, gb):
    in_maps = _make_in_maps(x, Ws, bs, pW, pb, gW, gb)
    out, _ = _run(in_maps)
    return out


def _install_ntff_hook():
    """Provide antenv.axon_hooks (absent in this image) so that
    run_bass_kernel_spmd(trace=True) can NTFF-profile via the axon .so."""
    try:
        from antenv.axon_hooks import get_axon_ntff_profile_hook  # noqa: F401

        return
    except ImportError:
        pass

    import contextlib
    import ctypes
    import sys
    import types

    so_path = "/opt/axon/libaxon_pjrt.so"
    lib = ctypes.CDLL(so_path)
    if not hasattr(lib, "axon_start_nrt_profile"):
        return
    lib.axon_start_nrt_profile.argtypes = [
        ctypes.POINTER(ctypes.c_int64),
        ctypes.c_size_t,
    ]
    lib.axon_start_nrt_profile.restype = ctypes.c_int64
    lib.axon_stop_nrt_profile.argtypes = [ctypes.c_char_p]
    lib.axon_stop_nrt_profile.restype = ctypes.c_int64

    @contextlib.contextmanager
    def _hook(output_dir, device_ids):
        import jax

        jax.devices()
        if device_ids:
            ids = (ctypes.c_int64 * len(device_ids))(*device_ids)
            rc = lib.axon_start_nrt_profile(ids, len(device_ids))
        else:
            rc = lib.axon_start_nrt_profile(None, 0)
        if rc != 0:
            raise RuntimeError(f"axon_start_nrt_profile rc={rc}")
        try:
            yield
        finally:
            n = lib.axon_stop_nrt_profile(str(output_dir).encode())
            print(f"[ntff] {n} file(s) written to {output_dir}")

    mod = types.ModuleType("antenv.axon_hooks")
    mod._hook = _hook
    mod.get_axon_ntff_profile_hook = lambda: mod._hook
    mod.set_axon_ntff_profile_hook = lambda h: setattr(mod, "_hook", h)
    sys.modules["antenv.axon_hooks"] = mod


def kernel_traced(x, Ws, bs, pW, pb, gW, gb, tmpdir=None):
    """Like kernel(), also returns the NTFF-profiled HW exec time (ns)."""
    _install_ntff_hook()
    in_maps = _make_in_maps(x, Ws, bs, pW, pb, gW, gb)
    out, res = _run(in_maps, trace=True, tmpdir=tmpdir)
    return out, res.exec_time_ns
